# revision 4
# baseline (speedup 1.0000x reference)
"""GNN message-passing (GraphSAGE-mean style) kernel for 8 TRN2 NeuronCores.

Self-contained: accepts the FULL inputs of the reference problem
(50000 nodes, 800000 edges, 128->128(x2 layers)->64) and returns the full
[50000, 64] output, distributing work across 8 NeuronCores internally.

Approach:
- Nodes sharded 8 ways (6250 + 22 pad = 6272/core, 49 blocks of 128).
- Edges sharded by src owner, grouped by (src block, dst int16-window),
  tile counts equalized across cores so one SPMD NEFF fits all.
- Per layer each core keeps the full fp16 m-table (m = h @ W_neigh) in its
  own HBM; per-edge messages fetched with dma_gather over 4 SWDGE queues.
- Segment-sum via PE matmuls with per-tile indicator matrices built on DVE.
- Dense updates in fp16 on PE with fp32 PSUM; relu/copies on ACT.
- Per-layer cross-core exchange via single-dest remote_dma_broadcast into
  XOR-slot SBUF staging, reordered into the node-ordered HBM table with
  dynamic-offset local DMAs (partition-id XOR register arithmetic).
"""
import numpy as np

import concourse.bacc as bacc
import concourse.bass as bass
import concourse.mybir as mybir
from concourse.tile import TileContext
from concourse.masks import make_identity
from concourse.tile_rust import add_dep_helper

N_NODES = 50000
N_EDGES = 800000
IN_DIM = 128
LAT = 128
OUT_DIM = 64

NCORE = 8
NPC_REAL = N_NODES // NCORE          # 6250
NBLK = 49                            # blocks of 128 per core
NPC = NBLK * 128                     # 6272 padded nodes per core
NTOT = NPC * NCORE                   # 50176 table rows
LO_ROWS = 32768
CH_T = 32                            # tiles per gather chunk
G = 13                               # blocks per exchange round
N_ROUNDS = 4
DELTA = [0, 1, 2, 3, 6, 7, 4, 5]     # HW slot -> delivered tpb delta

F16 = mybir.dt.float16
F32 = mybir.dt.float32
I16 = mybir.dt.int16


def preprocess(features, edge_list):
    src = edge_list[:, 0].astype(np.int64)
    dst = edge_list[:, 1].astype(np.int64)
    cnt = np.bincount(src, minlength=N_NODES).astype(np.float32)
    inv_deg_full = (1.0 / np.maximum(cnt, 1.0)).astype(np.float32)

    tpos = (dst // NPC_REAL) * NPC + (dst % NPC_REAL)
    owner = src // NPC_REAL
    srel_all = src % NPC_REAL
    blk_all = srel_all // 128

    order = np.lexsort((tpos, blk_all, owner))
    so, sb = owner[order], blk_all[order]
    sw = (tpos[order] >= LO_ROWS).astype(np.int64)
    st, sr = tpos[order], srel_all[order]
    key = ((so * NBLK) + sb) * 2 + sw
    bounds = np.searchsorted(key, np.arange(NCORE * NBLK * 2 + 1))
    per = {}
    for c in range(NCORE):
        for b in range(NBLK):
            for w in range(2):
                k = (c * NBLK + b) * 2 + w
                lo, hi = bounds[k], bounds[k + 1]
                per[(c, b, w)] = (st[lo:hi], sr[lo:hi])

    T = np.zeros((NBLK, 2), np.int64)
    for b in range(NBLK):
        for w in range(2):
            mx = max(len(per[(c, b, w)][0]) for c in range(NCORE))
            T[b, w] = max(1, -(-mx // 128))

    tiles = []
    for w in range(2):
        for b in range(NBLK):
            for i in range(T[b, w]):
                tiles.append((b, w, i))
    NT = len(tiles)

    chunks = []
    i = 0
    while i < NT:
        w = tiles[i][1]
        j = i
        while j < NT and tiles[j][1] == w and j - i < CH_T:
            j += 1
        chunks.append((i, j - i, w))
        i = j

    gidx = np.zeros((NCORE, 128, NT * 8), np.int16)
    srcrel = np.full((NCORE, 128, NT), -1.0, np.float16)
    for c in range(NCORE):
        for t, (b, w, i) in enumerate(tiles):
            pos, rel = per[(c, b, w)]
            seg = pos[i * 128:(i + 1) * 128]
            relseg = rel[i * 128:(i + 1) * 128]
            n = len(seg)
            idx = np.zeros(128, np.int64)
            idx[:n] = seg - (LO_ROWS if w else 0)
            col = np.full(128, -1.0, np.float32)
            col[:n] = relseg - b * 128
            a = idx.astype(np.int16).reshape(8, 16).T
            gidx[c, :, t * 8:(t + 1) * 8] = np.tile(a, (8, 1))
            srcrel[c, :, t] = col.astype(np.float16)

    featT = np.zeros((NCORE, 128, NPC), np.float32)
    invd = np.ones((NCORE, 128, NBLK), np.float32)
    for c in range(NCORE):
        f = features[c * NPC_REAL:(c + 1) * NPC_REAL]
        featT[c, :, :NPC_REAL] = f.T
        col = np.ones(NPC, np.float32)
        col[:NPC_REAL] = inv_deg_full[c * NPC_REAL:(c + 1) * NPC_REAL]
        invd[c] = col.reshape(NBLK, 128).T

    meta = dict(tiles=tiles, chunks=chunks, T=T, NT=NT)
    return meta, featT, invd, gidx, srcrel


def build(meta):
    tiles, chunks, T, NT = meta["tiles"], meta["chunks"], meta["T"], meta["NT"]
    nc = bacc.Bacc("TRN2", target_bir_lowering=False, debug=False,
                   enable_asserts=True, num_devices=NCORE, num_swdge_queues=4)

    featT_d = nc.dram_tensor("featT", [128, NPC], F32, kind="ExternalInput")
    wenc_d = nc.dram_tensor("wenc", [IN_DIM, LAT], F32, kind="ExternalInput")
    benc_d = nc.dram_tensor("benc", [1, LAT], F32, kind="ExternalInput")
    ws_d = [nc.dram_tensor(f"ws{l}", [LAT, LAT], F16, kind="ExternalInput") for l in range(2)]
    wn_d = [nc.dram_tensor(f"wn{l}", [LAT, LAT], F16, kind="ExternalInput") for l in range(2)]
    bc_d = [nc.dram_tensor(f"bc{l}", [1, LAT], F16, kind="ExternalInput") for l in range(2)]
    wout_d = nc.dram_tensor("wout", [LAT, OUT_DIM], F16, kind="ExternalInput")
    bout_d = nc.dram_tensor("bout", [1, OUT_DIM], F16, kind="ExternalInput")
    invd_d = nc.dram_tensor("invd", [128, NBLK], F32, kind="ExternalInput")
    gidx_d = nc.dram_tensor("gidx", [128, NT * 8], I16, kind="ExternalInput")
    srcrel_d = nc.dram_tensor("srcrel", [128, NT], F16, kind="ExternalInput")
    out_d = nc.dram_tensor("outp", [NPC, OUT_DIM], F32, kind="ExternalOutput")
    table_d = nc.dram_tensor("table", [NTOT, LAT], F16, kind="Internal")

    with TileContext(nc) as tc:
        with tc.tile_pool(name="consts", bufs=1) as cpool, \
             tc.tile_pool(name="agg", bufs=1) as apool, \
             tc.tile_pool(name="ht", bufs=1) as hpool, \
             tc.tile_pool(name="stage", bufs=1) as spool, \
             tc.tile_pool(name="msg", bufs=8) as mpool, \
             tc.tile_pool(name="idx", bufs=8) as ipool, \
             tc.tile_pool(name="stile", bufs=8) as stp, \
             tc.tile_pool(name="tmp", bufs=4) as tpool, \
             tc.tile_pool(name="outb", bufs=3) as opool, \
             tc.tile_pool(name="pseg", bufs=4, space="PSUM") as pseg, \
             tc.tile_pool(name="pdns", bufs=3, space="PSUM") as pdns, \
             tc.tile_pool(name="ptrn", bufs=1, space="PSUM") as ptrn:

            # ---- constants ----
            wenc = cpool.tile([IN_DIM, LAT], F32)
            nc.sync.dma_start(out=wenc[:], in_=wenc_d[:])
            benc = cpool.tile([1, LAT], F32)
            nc.sync.dma_start(out=benc[:], in_=benc_d[:])
            ws, wn, bc = [], [], []
            for l in range(2):
                w_ = cpool.tile([LAT, LAT], F16, tag=f"ws{l}")
                nc.sync.dma_start(out=w_[:], in_=ws_d[l][:])
                ws.append(w_)
                w2 = cpool.tile([LAT, LAT], F16, tag=f"wn{l}")
                nc.sync.dma_start(out=w2[:], in_=wn_d[l][:])
                wn.append(w2)
                b_ = cpool.tile([1, LAT], F16, tag=f"bc{l}")
                nc.sync.dma_start(out=b_[:], in_=bc_d[l][:])
                bc.append(b_)
            wout = cpool.tile([LAT, OUT_DIM], F16)
            nc.sync.dma_start(out=wout[:], in_=wout_d[:])
            bout = cpool.tile([1, OUT_DIM], F16)
            nc.sync.dma_start(out=bout[:], in_=bout_d[:])
            invd = cpool.tile([128, NBLK], F32)
            nc.sync.dma_start(out=invd[:], in_=invd_d[:])
            srcrel = cpool.tile([128, NT], F16)
            nc.gpsimd.dma_start(out=srcrel[:], in_=srcrel_d[:])
            ones16 = cpool.tile([1, 128], F16)
            nc.vector.memset(ones16[:], 1.0)
            ones32 = cpool.tile([1, 128], F32)
            nc.vector.memset(ones32[:], 1.0)
            iota32 = cpool.tile([128, 128], F32)
            nc.gpsimd.iota(iota32[:], [[1, 128]], base=0, channel_multiplier=0,
                           allow_small_or_imprecise_dtypes=True)
            iota = cpool.tile([128, 128], F16)
            nc.vector.tensor_copy(out=iota[:], in_=iota32[:])
            ident = cpool.tile([128, 128], F16)
            make_identity(nc, ident[:])

            agg = apool.tile([128, NBLK, 128], F32)
            h_T = hpool.tile([128, NPC], F16)
            h_stage = spool.tile([128, NBLK, 128], F16)
            staging = spool.tile([128, 2, NCORE, G * 128], F16)

            gsem = [nc.alloc_semaphore(f"gsem{q}") for q in range(4)]
            rsem = [nc.alloc_semaphore(f"rsem{k}") for k in range(NCORE)]
            lsem = nc.alloc_semaphore("lsem")
            prep_sem = nc.alloc_semaphore("prep_sem")
            wsem = nc.alloc_semaphore("wsem")
            rnd = [0]

            with tc.tile_critical():
                nc.gpsimd.bir_kernel_barrier_wait([[i for i in range(NCORE)]])

            def exchange():
                with tc.tile_critical():
                    for r in range(N_ROUNDS):
                        g0 = r * G
                        g1 = min(NBLK, g0 + G)
                        ng = g1 - g0
                        rho = rnd[0]
                        par = rho % 2
                        if rho > 0:
                            nc.gpsimd.wait_ge(wsem, 128 * rho)
                        for k in range(NCORE):
                            rdests = [None] * 8
                            rdests[k] = (0, k)
                            nc.gpsimd.remote_dma_broadcast(
                                out_ap=staging[:, par, k, :ng * 128],
                                in_ap=h_stage[:, g0:g1].rearrange(
                                    "p g f -> p (g f)"),
                                remote_sem=rsem[k], local_sem=lsem,
                                rdests=rdests, queue_num=k % 4,
                            ).then_inc(prep_sem, 1)
                        nc.gpsimd.wait_ge(prep_sem, 8 * (rho + 1))
                        for q in range(4):
                            nc.gpsimd.trigger_dma(count=2, queue_num=q)
                        pid = nc.sync.partition_id()
                        for k in range(NCORE):
                            nc.sync.wait_ge(rsem[k], 2 * (rho + 1))
                            srcc = nc.sync.alloc_register(f"sc{rho}_{k}")
                            nc.sync.reg_alu(srcc, pid, DELTA[k],
                                            mybir.AluOpType.bitwise_xor)
                            srcv = nc.sync.snap(srcc, donate=True,
                                                min_val=0, max_val=7)
                            row0 = srcv * NPC + (g0 * 128)
                            nc.sync.dma_start(
                                out=table_d[bass.ds(row0, ng * 128), :].rearrange(
                                    "(g p) f -> p g f", p=128),
                                in_=staging[:, par, k, :ng * 128].rearrange(
                                    "p (g f) -> p g f", f=LAT),
                            ).then_inc(wsem, 16)
                        rnd[0] += 1
                    nc.sync.wait_ge(wsem, 128 * rnd[0])

            # ---- encoder ----
            scope_enc = nc.named_scope("encoder"); scope_enc.__enter__()
            for b in range(NBLK):
                bs = slice(b * 128, (b + 1) * 128)
                fb = tpool.tile([128, 128], F32, tag="fb")
                nc.scalar.dma_start(out=fb[:], in_=featT_d[:, bs])
                p1 = pdns.tile([128, 128], F32, tag="pd")
                nc.tensor.matmul(out=p1[:], lhsT=benc[:], rhs=ones32[:],
                                 start=True, stop=False)
                nc.tensor.matmul(out=p1[:], lhsT=wenc[:], rhs=fb[:],
                                 start=False, stop=True)
                nc.scalar.activation(out=h_T[:, bs], in_=p1[:],
                                     func=mybir.ActivationFunctionType.Copy)
                pm = pdns.tile([128, 128], F32, tag="pd")
                nc.tensor.matmul(out=pm[:], lhsT=h_T[:, bs], rhs=wn[0][:],
                                 start=True, stop=True)
                nc.scalar.activation(out=h_stage[:, b], in_=pm[:],
                                     func=mybir.ActivationFunctionType.Copy)

            scope_enc.__exit__(None, None, None)
            with nc.named_scope("exchange0"):
                exchange()

            # ---- layers ----
            for l in range(2):
                # segment phase
                cur_psum = None
                for ci, (t0, nt, w) in enumerate(chunks):
                    it = ipool.tile([128, CH_T * 8], I16)
                    nc.scalar.dma_start(out=it[:, :nt * 8],
                                        in_=gidx_d[:, t0 * 8:(t0 + nt) * 8])
                    msg = mpool.tile([128, CH_T, 128], F16)
                    src_ap = table_d[0:LO_ROWS, :] if w == 0 else table_d[LO_ROWS:NTOT, :]
                    nc.gpsimd.dma_gather(
                        out_ap=msg[:, :nt, :], in_ap=src_ap,
                        idxs_ap=it[:, :nt * 8],
                        num_idxs=128 * nt, num_idxs_reg=128 * nt,
                        elem_size=LAT, single_packet=False,
                        queue_num=ci % 4)
                    SW = 4   # S tiles built per DVE op
                    sbuilt = {}
                    for j0 in range(0, nt, SW):
                        jn = min(SW, nt - j0)
                        St = stp.tile([128, SW, 128], F16)
                        nc.vector.tensor_tensor(
                            out=St[:, :jn, :],
                            in0=iota[:].unsqueeze(1).to_broadcast([128, jn, 128]),
                            in1=srcrel[:, t0 + j0:t0 + j0 + jn].to_broadcast(
                                [128, jn, 128]),
                            op=mybir.AluOpType.is_equal)
                        sbuilt[j0] = St
                    for j in range(nt):
                        t = t0 + j
                        b, w_, i = tiles[t]
                        St = sbuilt[j - j % SW]
                        if i == 0:
                            cur_psum = pseg.tile([128, 128], F32, tag="pg")
                        last = (i == T[b, w_] - 1)
                        nc.tensor.matmul(out=cur_psum[:], lhsT=St[:, j % SW, :],
                                         rhs=msg[:, j, :],
                                         start=(i == 0), stop=last)
                        if last:
                            if w_ == 0:
                                nc.vector.tensor_copy(out=agg[:, b], in_=cur_psum[:])
                            else:
                                nc.vector.tensor_tensor(
                                    out=agg[:, b], in0=agg[:, b], in1=cur_psum[:],
                                    op=mybir.AluOpType.add)
                # dense phase
                for b in range(NBLK):
                    bs = slice(b * 128, (b + 1) * 128)
                    pd = pdns.tile([128, 128], F32, tag="pd")
                    nc.tensor.matmul(out=pd[:], lhsT=ones16[:], rhs=bc[l][:],
                                     start=True, stop=False)
                    nc.tensor.matmul(out=pd[:], lhsT=h_T[:, bs], rhs=ws[l][:],
                                     start=False, stop=True)
                    tmp = tpool.tile([128, 128], F32, tag="tmp")
                    nc.scalar.activation(out=tmp[:], in_=agg[:, b],
                                         func=mybir.ActivationFunctionType.Copy,
                                         scale=invd[:, b:b + 1])
                    tmp2 = tpool.tile([128, 128], F32, tag="tmp2")
                    nc.vector.tensor_tensor(out=tmp2[:], in0=tmp[:], in1=pd[:],
                                            op=mybir.AluOpType.add)
                    hn = tpool.tile([128, 128], F16, tag="hn")
                    nc.scalar.activation(out=hn[:], in_=tmp2[:],
                                         func=mybir.ActivationFunctionType.Relu)
                    pt = ptrn.tile([128, 128], F16, tag="pt")
                    nc.tensor.transpose(out=pt[:], in_=hn[:], identity=ident[:])
                    nc.scalar.activation(out=h_T[:, bs], in_=pt[:],
                                         func=mybir.ActivationFunctionType.Copy)
                    if l == 0:
                        pm = pdns.tile([128, 128], F32, tag="pd")
                        nc.tensor.matmul(out=pm[:], lhsT=h_T[:, bs], rhs=wn[1][:],
                                         start=True, stop=True)
                        nc.scalar.activation(out=h_stage[:, b], in_=pm[:],
                                             func=mybir.ActivationFunctionType.Copy)
                if l == 0:
                    with nc.named_scope("exchange1"):
                        exchange()

            # ---- output ----
            for b in range(NBLK):
                bs = slice(b * 128, (b + 1) * 128)
                po = pdns.tile([128, OUT_DIM], F32, tag="pd")
                nc.tensor.matmul(out=po[:], lhsT=ones16[:], rhs=bout[:],
                                 start=True, stop=False)
                nc.tensor.matmul(out=po[:], lhsT=h_T[:, bs], rhs=wout[:],
                                 start=False, stop=True)
                ob = opool.tile([128, OUT_DIM], F32)
                nc.scalar.activation(out=ob[:], in_=po[:],
                                     func=mybir.ActivationFunctionType.Copy)
                nc.sync.dma_start(out=out_d[bs, :], in_=ob[:])

    nc.compile()
    return nc


def make_in_maps(inputs):
    features = np.asarray(inputs["features"], np.float32)
    edge_list = np.asarray(inputs["edge_list"])
    meta, featT, invd, gidx, srcrel = preprocess(features, edge_list)
    w16 = lambda x: np.asarray(x, np.float16)
    in_maps = []
    for c in range(NCORE):
        in_maps.append(dict(
            featT=featT[c], invd=invd[c], gidx=gidx[c], srcrel=srcrel[c],
            wenc=np.asarray(inputs["W_enc"], np.float32),
            benc=np.asarray(inputs["b_enc"], np.float32).reshape(1, LAT),
            ws0=w16(inputs["W_self"][0]), ws1=w16(inputs["W_self"][1]),
            wn0=w16(inputs["W_neigh"][0]), wn1=w16(inputs["W_neigh"][1]),
            bc0=w16(inputs["b_comb"][0]).reshape(1, LAT),
            bc1=w16(inputs["b_comb"][1]).reshape(1, LAT),
            wout=w16(inputs["W_out"]),
            bout=w16(inputs["b_out"]).reshape(1, OUT_DIM),
        ))
    return meta, in_maps


def assemble(results):
    outs = [results[c]["outp"][:NPC_REAL] for c in range(NCORE)]
    return np.concatenate(outs, axis=0)


def kernel(**inputs):
    """Full-input entry point: shard, compile, run on 8 cores, gather."""
    from concourse import bass_utils
    meta, in_maps = make_in_maps(inputs)
    nc = build(meta)
    res = bass_utils.run_bass_kernel_spmd(nc, in_maps, core_ids=list(range(NCORE)))
    return assemble(res.results)



# revision 6
# speedup vs baseline: 1.1289x; 1.1289x over previous
"""GNN message-passing (GraphSAGE-mean style) kernel for 8 TRN2 NeuronCores.

Self-contained: accepts the FULL inputs of the reference problem
(50000 nodes, 800000 edges, 128->128(x2 layers)->64) and returns the full
[50000, 64] output, distributing work across 8 NeuronCores internally.

Approach:
- Nodes sharded 8 ways (6250 + 22 pad = 6272/core, 49 blocks of 128).
- Edges sharded by src owner, grouped by (src block, dst int16-window),
  tile counts equalized across cores so one SPMD NEFF fits all.
- Per layer each core keeps the full fp16 m-table (m = h @ W_neigh) in its
  own HBM; per-edge messages fetched with dma_gather over 4 SWDGE queues.
- Segment-sum via PE matmuls with per-tile indicator matrices built on DVE.
- Dense updates in fp16 on PE with fp32 PSUM; relu/copies on ACT.
- Per-layer cross-core exchange via single-dest remote_dma_broadcast into
  XOR-slot SBUF staging, reordered into the node-ordered HBM table with
  dynamic-offset local DMAs (partition-id XOR register arithmetic).
"""
import numpy as np

import concourse.bacc as bacc
import concourse.bass as bass
import concourse.mybir as mybir
from concourse.tile import TileContext
from concourse.masks import make_identity
from concourse.tile_rust import add_dep_helper

N_NODES = 50000
N_EDGES = 800000
IN_DIM = 128
LAT = 128
OUT_DIM = 64

NCORE = 8
NPC_REAL = N_NODES // NCORE          # 6250
NBLK = 49                            # blocks of 128 per core
NPC = NBLK * 128                     # 6272 padded nodes per core
NTOT = NPC * NCORE                   # 50176 table rows
LO_ROWS = 32768
CH_T = 32                            # tiles per gather chunk
G = 25                               # blocks per exchange round
N_ROUNDS = 2
DELTA = [0, 1, 2, 3, 6, 7, 4, 5]     # HW slot -> delivered tpb delta

F16 = mybir.dt.float16
F32 = mybir.dt.float32
I16 = mybir.dt.int16


def preprocess(features, edge_list):
    src = edge_list[:, 0].astype(np.int64)
    dst = edge_list[:, 1].astype(np.int64)
    cnt = np.bincount(src, minlength=N_NODES).astype(np.float32)
    inv_deg_full = (1.0 / np.maximum(cnt, 1.0)).astype(np.float32)

    tpos = (dst // NPC_REAL) * NPC + (dst % NPC_REAL)
    owner = src // NPC_REAL
    srel_all = src % NPC_REAL
    blk_all = srel_all // 128

    order = np.lexsort((tpos, blk_all, owner))
    so, sb = owner[order], blk_all[order]
    sw = (tpos[order] >= LO_ROWS).astype(np.int64)
    st, sr = tpos[order], srel_all[order]
    key = ((so * NBLK) + sb) * 2 + sw
    bounds = np.searchsorted(key, np.arange(NCORE * NBLK * 2 + 1))
    per = {}
    for c in range(NCORE):
        for b in range(NBLK):
            for w in range(2):
                k = (c * NBLK + b) * 2 + w
                lo, hi = bounds[k], bounds[k + 1]
                per[(c, b, w)] = (st[lo:hi], sr[lo:hi])

    T = np.zeros((NBLK, 2), np.int64)
    for b in range(NBLK):
        for w in range(2):
            mx = max(len(per[(c, b, w)][0]) for c in range(NCORE))
            T[b, w] = max(1, -(-mx // 128))

    tiles = []
    for w in range(2):
        for b in range(NBLK):
            for i in range(T[b, w]):
                tiles.append((b, w, i))
    NT = len(tiles)

    chunks = []
    i = 0
    while i < NT:
        w = tiles[i][1]
        j = i
        while j < NT and tiles[j][1] == w and j - i < CH_T:
            j += 1
        chunks.append((i, j - i, w))
        i = j

    gidx = np.zeros((NCORE, 128, NT * 8), np.int16)
    srcrel = np.full((NCORE, 128, NT), -1.0, np.float16)
    for c in range(NCORE):
        for t, (b, w, i) in enumerate(tiles):
            pos, rel = per[(c, b, w)]
            seg = pos[i * 128:(i + 1) * 128]
            relseg = rel[i * 128:(i + 1) * 128]
            n = len(seg)
            idx = np.zeros(128, np.int64)
            idx[:n] = seg - (LO_ROWS if w else 0)
            col = np.full(128, -1.0, np.float32)
            col[:n] = relseg - b * 128
            a = idx.astype(np.int16).reshape(8, 16).T
            gidx[c, :, t * 8:(t + 1) * 8] = np.tile(a, (8, 1))
            srcrel[c, :, t] = col.astype(np.float16)

    featT = np.zeros((NCORE, 128, NPC), np.float32)
    invd = np.ones((NCORE, 128, NBLK), np.float32)
    for c in range(NCORE):
        f = features[c * NPC_REAL:(c + 1) * NPC_REAL]
        featT[c, :, :NPC_REAL] = f.T
        col = np.ones(NPC, np.float32)
        col[:NPC_REAL] = inv_deg_full[c * NPC_REAL:(c + 1) * NPC_REAL]
        invd[c] = col.reshape(NBLK, 128).T

    meta = dict(tiles=tiles, chunks=chunks, T=T, NT=NT)
    return meta, featT, invd, gidx, srcrel


def build(meta):
    tiles, chunks, T, NT = meta["tiles"], meta["chunks"], meta["T"], meta["NT"]
    nc = bacc.Bacc("TRN2", target_bir_lowering=False, debug=False,
                   enable_asserts=True, num_devices=NCORE, num_swdge_queues=4)

    featT_d = nc.dram_tensor("featT", [128, NPC], F32, kind="ExternalInput")
    wenc_d = nc.dram_tensor("wenc", [IN_DIM, LAT], F32, kind="ExternalInput")
    benc_d = nc.dram_tensor("benc", [1, LAT], F32, kind="ExternalInput")
    ws_d = [nc.dram_tensor(f"ws{l}", [LAT, LAT], F16, kind="ExternalInput") for l in range(2)]
    wn_d = [nc.dram_tensor(f"wn{l}", [LAT, LAT], F16, kind="ExternalInput") for l in range(2)]
    bc_d = [nc.dram_tensor(f"bc{l}", [1, LAT], F16, kind="ExternalInput") for l in range(2)]
    wout_d = nc.dram_tensor("wout", [LAT, OUT_DIM], F16, kind="ExternalInput")
    bout_d = nc.dram_tensor("bout", [1, OUT_DIM], F16, kind="ExternalInput")
    invd_d = nc.dram_tensor("invd", [128, NBLK], F32, kind="ExternalInput")
    gidx_d = nc.dram_tensor("gidx", [128, NT * 8], I16, kind="ExternalInput")
    srcrel_d = nc.dram_tensor("srcrel", [128, NT], F16, kind="ExternalInput")
    out_d = nc.dram_tensor("outp", [NPC, OUT_DIM], F32, kind="ExternalOutput")
    table_d = nc.dram_tensor("table", [NTOT, LAT], F16, kind="Internal")

    with TileContext(nc) as tc:
        with tc.tile_pool(name="consts", bufs=1) as cpool, \
             tc.tile_pool(name="agg", bufs=1) as apool, \
             tc.tile_pool(name="ht", bufs=1) as hpool, \
             tc.tile_pool(name="stage", bufs=1) as spool, \
             tc.tile_pool(name="msg", bufs=4) as mpool, \
             tc.tile_pool(name="idx", bufs=6) as ipool, \
             tc.tile_pool(name="stile", bufs=6) as stp, \
             tc.tile_pool(name="tmp", bufs=4) as tpool, \
             tc.tile_pool(name="outb", bufs=3) as opool, \
             tc.tile_pool(name="pseg", bufs=4, space="PSUM") as pseg, \
             tc.tile_pool(name="pdns", bufs=3, space="PSUM") as pdns, \
             tc.tile_pool(name="ptrn", bufs=1, space="PSUM") as ptrn:

            # ---- constants ----
            wenc = cpool.tile([IN_DIM, LAT], F32)
            nc.sync.dma_start(out=wenc[:], in_=wenc_d[:])
            benc = cpool.tile([1, LAT], F32)
            nc.sync.dma_start(out=benc[:], in_=benc_d[:])
            ws, wn, bc = [], [], []
            for l in range(2):
                w_ = cpool.tile([LAT, LAT], F16, tag=f"ws{l}")
                nc.sync.dma_start(out=w_[:], in_=ws_d[l][:])
                ws.append(w_)
                w2 = cpool.tile([LAT, LAT], F16, tag=f"wn{l}")
                nc.sync.dma_start(out=w2[:], in_=wn_d[l][:])
                wn.append(w2)
                b_ = cpool.tile([1, LAT], F16, tag=f"bc{l}")
                nc.sync.dma_start(out=b_[:], in_=bc_d[l][:])
                bc.append(b_)
            wout = cpool.tile([LAT, OUT_DIM], F16)
            nc.sync.dma_start(out=wout[:], in_=wout_d[:])
            bout = cpool.tile([1, OUT_DIM], F16)
            nc.sync.dma_start(out=bout[:], in_=bout_d[:])
            invd = cpool.tile([128, NBLK], F32)
            nc.sync.dma_start(out=invd[:], in_=invd_d[:])
            srcrel = cpool.tile([128, NT], F16)
            nc.gpsimd.dma_start(out=srcrel[:], in_=srcrel_d[:])
            ones16 = cpool.tile([1, 128], F16)
            nc.vector.memset(ones16[:], 1.0)
            ones32 = cpool.tile([1, 128], F32)
            nc.vector.memset(ones32[:], 1.0)
            iota32 = cpool.tile([128, 128], F32)
            nc.gpsimd.iota(iota32[:], [[1, 128]], base=0, channel_multiplier=0,
                           allow_small_or_imprecise_dtypes=True)
            iota = cpool.tile([128, 128], F16)
            nc.vector.tensor_copy(out=iota[:], in_=iota32[:])
            ident = cpool.tile([128, 128], F16)
            make_identity(nc, ident[:])

            agg = apool.tile([128, NBLK, 128], F32)
            h_T = hpool.tile([128, NPC], F16)
            h_stage = spool.tile([128, NBLK, 128], F16)
            staging = spool.tile([128, 2, NCORE, G * 128], F16)

            gsem = [nc.alloc_semaphore(f"gsem{q}") for q in range(4)]
            rsem = [nc.alloc_semaphore(f"rsem{k}") for k in range(NCORE)]
            lsem = nc.alloc_semaphore("lsem")
            prep_sem = nc.alloc_semaphore("prep_sem")
            wsem = nc.alloc_semaphore("wsem")
            rnd = [0]

            with tc.tile_critical():
                nc.gpsimd.bir_kernel_barrier_wait([[i for i in range(NCORE)]])

            def exchange():
                with tc.tile_critical():
                    for r in range(N_ROUNDS):
                        g0 = r * G
                        g1 = min(NBLK, g0 + G)
                        ng = g1 - g0
                        rho = rnd[0]
                        par = rho % 2
                        if rho > 0:
                            nc.gpsimd.wait_ge(wsem, 128 * rho)
                        for k in range(NCORE):
                            rdests = [None] * 8
                            rdests[k] = (0, k)
                            nc.gpsimd.remote_dma_broadcast(
                                out_ap=staging[:, par, k, :ng * 128],
                                in_ap=h_stage[:, g0:g1].rearrange(
                                    "p g f -> p (g f)"),
                                remote_sem=rsem[k], local_sem=lsem,
                                rdests=rdests, queue_num=k % 4,
                            ).then_inc(prep_sem, 1)
                        nc.gpsimd.wait_ge(prep_sem, 8 * (rho + 1))
                        for q in range(4):
                            nc.gpsimd.trigger_dma(count=2, queue_num=q)
                        pid = nc.sync.partition_id()
                        for k in range(NCORE):
                            nc.sync.wait_ge(rsem[k], 2 * (rho + 1))
                            srcc = nc.sync.alloc_register(f"sc{rho}_{k}")
                            nc.sync.reg_alu(srcc, pid, DELTA[k],
                                            mybir.AluOpType.bitwise_xor)
                            srcv = nc.sync.snap(srcc, donate=True,
                                                min_val=0, max_val=7)
                            row0 = srcv * NPC + (g0 * 128)
                            nc.sync.dma_start(
                                out=table_d[bass.ds(row0, ng * 128), :].rearrange(
                                    "(g p) f -> p g f", p=128),
                                in_=staging[:, par, k, :ng * 128].rearrange(
                                    "p (g f) -> p g f", f=LAT),
                            ).then_inc(wsem, 16)
                        rnd[0] += 1
                    nc.sync.wait_ge(wsem, 128 * rnd[0])

            # ---- encoder ----
            scope_enc = nc.named_scope("encoder"); scope_enc.__enter__()
            for b in range(NBLK):
                bs = slice(b * 128, (b + 1) * 128)
                fb = tpool.tile([128, 128], F32, tag="fb")
                nc.scalar.dma_start(out=fb[:], in_=featT_d[:, bs])
                p1 = pdns.tile([128, 128], F32, tag="pd")
                nc.tensor.matmul(out=p1[:], lhsT=benc[:], rhs=ones32[:],
                                 start=True, stop=False)
                nc.tensor.matmul(out=p1[:], lhsT=wenc[:], rhs=fb[:],
                                 start=False, stop=True)
                nc.scalar.activation(out=h_T[:, bs], in_=p1[:],
                                     func=mybir.ActivationFunctionType.Copy)
                pm = pdns.tile([128, 128], F32, tag="pd")
                nc.tensor.matmul(out=pm[:], lhsT=h_T[:, bs], rhs=wn[0][:],
                                 start=True, stop=True)
                nc.scalar.activation(out=h_stage[:, b], in_=pm[:],
                                     func=mybir.ActivationFunctionType.Copy)

            scope_enc.__exit__(None, None, None)
            with nc.named_scope("exchange0"):
                exchange()

            # ---- layers ----
            for l in range(2):
                # segment phase
                cur_psum = None
                for ci, (t0, nt, w) in enumerate(chunks):
                    it = ipool.tile([128, CH_T * 8], I16)
                    nc.scalar.dma_start(out=it[:, :nt * 8],
                                        in_=gidx_d[:, t0 * 8:(t0 + nt) * 8])
                    msg = mpool.tile([128, CH_T, 128], F16)
                    src_ap = table_d[0:LO_ROWS, :] if w == 0 else table_d[LO_ROWS:NTOT, :]
                    nc.gpsimd.dma_gather(
                        out_ap=msg[:, :nt, :], in_ap=src_ap,
                        idxs_ap=it[:, :nt * 8],
                        num_idxs=128 * nt, num_idxs_reg=128 * nt,
                        elem_size=LAT, single_packet=False,
                        queue_num=ci % 4)
                    SW = 4   # S tiles built per DVE op
                    sbuilt = {}
                    for j0 in range(0, nt, SW):
                        jn = min(SW, nt - j0)
                        St = stp.tile([128, SW, 128], F16)
                        nc.vector.tensor_tensor(
                            out=St[:, :jn, :],
                            in0=iota[:].unsqueeze(1).to_broadcast([128, jn, 128]),
                            in1=srcrel[:, t0 + j0:t0 + j0 + jn].to_broadcast(
                                [128, jn, 128]),
                            op=mybir.AluOpType.is_equal)
                        sbuilt[j0] = St
                    for j in range(nt):
                        t = t0 + j
                        b, w_, i = tiles[t]
                        St = sbuilt[j - j % SW]
                        if i == 0:
                            cur_psum = pseg.tile([128, 128], F32, tag="pg")
                        last = (i == T[b, w_] - 1)
                        nc.tensor.matmul(out=cur_psum[:], lhsT=St[:, j % SW, :],
                                         rhs=msg[:, j, :],
                                         start=(i == 0), stop=last)
                        if last:
                            if w_ == 0:
                                nc.vector.tensor_copy(out=agg[:, b], in_=cur_psum[:])
                            else:
                                nc.vector.tensor_tensor(
                                    out=agg[:, b], in0=agg[:, b], in1=cur_psum[:],
                                    op=mybir.AluOpType.add)
                # dense phase
                for b in range(NBLK):
                    bs = slice(b * 128, (b + 1) * 128)
                    pd = pdns.tile([128, 128], F32, tag="pd")
                    nc.tensor.matmul(out=pd[:], lhsT=ones16[:], rhs=bc[l][:],
                                     start=True, stop=False)
                    nc.tensor.matmul(out=pd[:], lhsT=h_T[:, bs], rhs=ws[l][:],
                                     start=False, stop=True)
                    tmp = tpool.tile([128, 128], F32, tag="tmp")
                    nc.scalar.activation(out=tmp[:], in_=agg[:, b],
                                         func=mybir.ActivationFunctionType.Copy,
                                         scale=invd[:, b:b + 1])
                    tmp2 = tpool.tile([128, 128], F32, tag="tmp2")
                    nc.vector.tensor_tensor(out=tmp2[:], in0=tmp[:], in1=pd[:],
                                            op=mybir.AluOpType.add)
                    hn = tpool.tile([128, 128], F16, tag="hn")
                    nc.scalar.activation(out=hn[:], in_=tmp2[:],
                                         func=mybir.ActivationFunctionType.Relu)
                    pt = ptrn.tile([128, 128], F16, tag="pt")
                    nc.tensor.transpose(out=pt[:], in_=hn[:], identity=ident[:])
                    nc.scalar.activation(out=h_T[:, bs], in_=pt[:],
                                         func=mybir.ActivationFunctionType.Copy)
                    if l == 0:
                        pm = pdns.tile([128, 128], F32, tag="pd")
                        nc.tensor.matmul(out=pm[:], lhsT=h_T[:, bs], rhs=wn[1][:],
                                         start=True, stop=True)
                        nc.scalar.activation(out=h_stage[:, b], in_=pm[:],
                                             func=mybir.ActivationFunctionType.Copy)
                if l == 0:
                    with nc.named_scope("exchange1"):
                        exchange()

            # ---- output ----
            for b in range(NBLK):
                bs = slice(b * 128, (b + 1) * 128)
                po = pdns.tile([128, OUT_DIM], F32, tag="pd")
                nc.tensor.matmul(out=po[:], lhsT=ones16[:], rhs=bout[:],
                                 start=True, stop=False)
                nc.tensor.matmul(out=po[:], lhsT=h_T[:, bs], rhs=wout[:],
                                 start=False, stop=True)
                ob = opool.tile([128, OUT_DIM], F32)
                nc.scalar.activation(out=ob[:], in_=po[:],
                                     func=mybir.ActivationFunctionType.Copy)
                nc.sync.dma_start(out=out_d[bs, :], in_=ob[:])

    nc.compile()
    return nc


def make_in_maps(inputs):
    features = np.asarray(inputs["features"], np.float32)
    edge_list = np.asarray(inputs["edge_list"])
    meta, featT, invd, gidx, srcrel = preprocess(features, edge_list)
    w16 = lambda x: np.asarray(x, np.float16)
    in_maps = []
    for c in range(NCORE):
        in_maps.append(dict(
            featT=featT[c], invd=invd[c], gidx=gidx[c], srcrel=srcrel[c],
            wenc=np.asarray(inputs["W_enc"], np.float32),
            benc=np.asarray(inputs["b_enc"], np.float32).reshape(1, LAT),
            ws0=w16(inputs["W_self"][0]), ws1=w16(inputs["W_self"][1]),
            wn0=w16(inputs["W_neigh"][0]), wn1=w16(inputs["W_neigh"][1]),
            bc0=w16(inputs["b_comb"][0]).reshape(1, LAT),
            bc1=w16(inputs["b_comb"][1]).reshape(1, LAT),
            wout=w16(inputs["W_out"]),
            bout=w16(inputs["b_out"]).reshape(1, OUT_DIM),
        ))
    return meta, in_maps


def assemble(results):
    outs = [results[c]["outp"][:NPC_REAL] for c in range(NCORE)]
    return np.concatenate(outs, axis=0)


def kernel(**inputs):
    """Full-input entry point: shard, compile, run on 8 cores, gather."""
    from concourse import bass_utils
    meta, in_maps = make_in_maps(inputs)
    nc = build(meta)
    res = bass_utils.run_bass_kernel_spmd(nc, in_maps, core_ids=list(range(NCORE)))
    return assemble(res.results)



# revision 8
# speedup vs baseline: 1.1825x; 1.0475x over previous
"""GNN message-passing (GraphSAGE-mean style) kernel for 8 TRN2 NeuronCores.

Self-contained: accepts the FULL inputs of the reference problem
(50000 nodes, 800000 edges, 128->128(x2 layers)->64) and returns the full
[50000, 64] output, distributing work across 8 NeuronCores internally.

Approach:
- Nodes sharded 8 ways (6250 + 22 pad = 6272/core, 49 blocks of 128).
- Edges sharded by src owner, grouped by (src block, dst int16-window),
  tile counts equalized across cores so one SPMD NEFF fits all.
- Per layer each core keeps the full fp16 m-table (m = h @ W_neigh) in its
  own HBM; per-edge messages fetched with dma_gather over 4 SWDGE queues.
- Segment-sum via PE matmuls with per-tile indicator matrices built on DVE.
- Dense updates in fp16 on PE with fp32 PSUM; relu/copies on ACT.
- Per-layer cross-core exchange via single-dest remote_dma_broadcast into
  XOR-slot SBUF staging, reordered into the node-ordered HBM table with
  dynamic-offset local DMAs (partition-id XOR register arithmetic).
"""
import numpy as np

import concourse.bacc as bacc
import concourse.bass as bass
import concourse.mybir as mybir
from concourse.tile import TileContext
from concourse.masks import make_identity
from concourse.tile_rust import add_dep_helper

N_NODES = 50000
N_EDGES = 800000
IN_DIM = 128
LAT = 128
OUT_DIM = 64

NCORE = 8
NPC_REAL = N_NODES // NCORE          # 6250
NBLK = 49                            # blocks of 128 per core
NPC = NBLK * 128                     # 6272 padded nodes per core
NTOT = NPC * NCORE                   # 50176 table rows
LO_ROWS = 32768
CH_T = 32                            # tiles per gather chunk
G = 25                               # blocks per exchange round
N_ROUNDS = 2
DELTA = [0, 1, 2, 3, 6, 7, 4, 5]     # HW slot -> delivered tpb delta

F16 = mybir.dt.float16
F32 = mybir.dt.float32
I16 = mybir.dt.int16


def preprocess(features, edge_list):
    src = edge_list[:, 0].astype(np.int64)
    dst = edge_list[:, 1].astype(np.int64)
    cnt = np.bincount(src, minlength=N_NODES).astype(np.float32)
    inv_deg_full = (1.0 / np.maximum(cnt, 1.0)).astype(np.float32)

    tpos = (dst // NPC_REAL) * NPC + (dst % NPC_REAL)
    owner = src // NPC_REAL
    srel_all = src % NPC_REAL
    blk_all = srel_all // 128

    order = np.lexsort((tpos, blk_all, owner))
    so, sb = owner[order], blk_all[order]
    sw = (tpos[order] >= LO_ROWS).astype(np.int64)
    st, sr = tpos[order], srel_all[order]
    key = ((so * NBLK) + sb) * 2 + sw
    bounds = np.searchsorted(key, np.arange(NCORE * NBLK * 2 + 1))
    per = {}
    for c in range(NCORE):
        for b in range(NBLK):
            for w in range(2):
                k = (c * NBLK + b) * 2 + w
                lo, hi = bounds[k], bounds[k + 1]
                per[(c, b, w)] = (st[lo:hi], sr[lo:hi])

    T = np.zeros((NBLK, 2), np.int64)
    for b in range(NBLK):
        for w in range(2):
            mx = max(len(per[(c, b, w)][0]) for c in range(NCORE))
            T[b, w] = max(1, -(-mx // 128))

    tiles = []
    for w in range(2):
        for b in range(NBLK):
            for i in range(T[b, w]):
                tiles.append((b, w, i))
    NT = len(tiles)

    chunks = []
    i = 0
    while i < NT:
        w = tiles[i][1]
        j = i
        while j < NT and tiles[j][1] == w and j - i < CH_T:
            j += 1
        chunks.append((i, j - i, w))
        i = j

    gidx = np.zeros((NCORE, 128, NT * 8), np.int16)
    srcrel = np.full((NCORE, 128, NT), -1.0, np.float16)
    for c in range(NCORE):
        for t, (b, w, i) in enumerate(tiles):
            pos, rel = per[(c, b, w)]
            seg = pos[i * 128:(i + 1) * 128]
            relseg = rel[i * 128:(i + 1) * 128]
            n = len(seg)
            idx = np.zeros(128, np.int64)
            idx[:n] = seg - (LO_ROWS if w else 0)
            col = np.full(128, -1.0, np.float32)
            col[:n] = relseg - b * 128
            a = idx.astype(np.int16).reshape(8, 16).T
            gidx[c, :, t * 8:(t + 1) * 8] = np.tile(a, (8, 1))
            srcrel[c, :, t] = col.astype(np.float16)

    featT = np.zeros((NCORE, 128, NPC), np.float32)
    invd = np.ones((NCORE, 128, NBLK), np.float32)
    for c in range(NCORE):
        f = features[c * NPC_REAL:(c + 1) * NPC_REAL]
        featT[c, :, :NPC_REAL] = f.T
        col = np.ones(NPC, np.float32)
        col[:NPC_REAL] = inv_deg_full[c * NPC_REAL:(c + 1) * NPC_REAL]
        invd[c] = col.reshape(NBLK, 128).T

    meta = dict(tiles=tiles, chunks=chunks, T=T, NT=NT)
    return meta, featT, invd, gidx, srcrel


def build(meta):
    tiles, chunks, T, NT = meta["tiles"], meta["chunks"], meta["T"], meta["NT"]
    nc = bacc.Bacc("TRN2", target_bir_lowering=False, debug=False,
                   enable_asserts=True, num_devices=NCORE, num_swdge_queues=4)

    featT_d = nc.dram_tensor("featT", [128, NPC], F32, kind="ExternalInput")
    wenc_d = nc.dram_tensor("wenc", [IN_DIM, LAT], F32, kind="ExternalInput")
    benc_d = nc.dram_tensor("benc", [1, LAT], F32, kind="ExternalInput")
    ws_d = [nc.dram_tensor(f"ws{l}", [LAT, LAT], F16, kind="ExternalInput") for l in range(2)]
    wn_d = [nc.dram_tensor(f"wn{l}", [LAT, LAT], F16, kind="ExternalInput") for l in range(2)]
    bc_d = [nc.dram_tensor(f"bc{l}", [1, LAT], F16, kind="ExternalInput") for l in range(2)]
    wout_d = nc.dram_tensor("wout", [LAT, OUT_DIM], F16, kind="ExternalInput")
    bout_d = nc.dram_tensor("bout", [1, OUT_DIM], F16, kind="ExternalInput")
    invd_d = nc.dram_tensor("invd", [128, NBLK], F32, kind="ExternalInput")
    gidx_d = nc.dram_tensor("gidx", [128, NT * 8], I16, kind="ExternalInput")
    srcrel_d = nc.dram_tensor("srcrel", [128, NT], F16, kind="ExternalInput")
    out_d = nc.dram_tensor("outp", [NPC, OUT_DIM], F32, kind="ExternalOutput")
    table_d = nc.dram_tensor("table", [NTOT, LAT], F16, kind="Internal")

    with TileContext(nc) as tc:
        with tc.tile_pool(name="consts", bufs=1) as cpool, \
             tc.tile_pool(name="agg", bufs=1) as apool, \
             tc.tile_pool(name="ht", bufs=1) as hpool, \
             tc.tile_pool(name="stage", bufs=1) as spool, \
             tc.tile_pool(name="msg", bufs=4) as mpool, \
             tc.tile_pool(name="idx", bufs=6) as ipool, \
             tc.tile_pool(name="stile", bufs=6) as stp, \
             tc.tile_pool(name="tmp", bufs=4) as tpool, \
             tc.tile_pool(name="outb", bufs=3) as opool, \
             tc.tile_pool(name="pseg", bufs=4, space="PSUM") as pseg, \
             tc.tile_pool(name="pdns", bufs=3, space="PSUM") as pdns, \
             tc.tile_pool(name="ptrn", bufs=1, space="PSUM") as ptrn:

            # ---- constants ----
            wenc = cpool.tile([IN_DIM, LAT], F32)
            nc.sync.dma_start(out=wenc[:], in_=wenc_d[:])
            benc = cpool.tile([1, LAT], F32)
            nc.sync.dma_start(out=benc[:], in_=benc_d[:])
            ws, wn, bc = [], [], []
            for l in range(2):
                w_ = cpool.tile([LAT, LAT], F16, tag=f"ws{l}")
                nc.sync.dma_start(out=w_[:], in_=ws_d[l][:])
                ws.append(w_)
                w2 = cpool.tile([LAT, LAT], F16, tag=f"wn{l}")
                nc.sync.dma_start(out=w2[:], in_=wn_d[l][:])
                wn.append(w2)
                b_ = cpool.tile([1, LAT], F16, tag=f"bc{l}")
                nc.sync.dma_start(out=b_[:], in_=bc_d[l][:])
                bc.append(b_)
            wout = cpool.tile([LAT, OUT_DIM], F16)
            nc.sync.dma_start(out=wout[:], in_=wout_d[:])
            bout = cpool.tile([1, OUT_DIM], F16)
            nc.sync.dma_start(out=bout[:], in_=bout_d[:])
            invd = cpool.tile([128, NBLK], F32)
            nc.sync.dma_start(out=invd[:], in_=invd_d[:])
            srcrel = cpool.tile([128, NT], F16)
            nc.gpsimd.dma_start(out=srcrel[:], in_=srcrel_d[:])
            ones16 = cpool.tile([1, 128], F16)
            nc.vector.memset(ones16[:], 1.0)
            ones32 = cpool.tile([1, 128], F32)
            nc.vector.memset(ones32[:], 1.0)
            iota32 = cpool.tile([128, 128], F32)
            nc.gpsimd.iota(iota32[:], [[1, 128]], base=0, channel_multiplier=0,
                           allow_small_or_imprecise_dtypes=True)
            iota = cpool.tile([128, 128], F16)
            nc.vector.tensor_copy(out=iota[:], in_=iota32[:])
            ident = cpool.tile([128, 128], F16)
            make_identity(nc, ident[:])

            agg = apool.tile([128, NBLK, 128], F32)
            h_T = hpool.tile([128, NPC], F16)
            h_stage = spool.tile([128, NBLK, 128], F16)
            staging = spool.tile([128, 2, NCORE, G * 128], F16)

            gsem = [nc.alloc_semaphore(f"gsem{q}") for q in range(4)]
            rsem = [nc.alloc_semaphore(f"rsem{k}") for k in range(NCORE)]
            lsem = nc.alloc_semaphore("lsem")
            prep_sem = nc.alloc_semaphore("prep_sem")
            wsem = nc.alloc_semaphore("wsem")
            rnd = [0]

            with tc.tile_critical():
                nc.gpsimd.bir_kernel_barrier_wait([[i for i in range(NCORE)]])

            def exchange():
                with tc.tile_critical():
                    for r in range(N_ROUNDS):
                        g0 = r * G
                        g1 = min(NBLK, g0 + G)
                        ng = g1 - g0
                        rho = rnd[0]
                        par = rho % 2
                        if rho > 0:
                            nc.gpsimd.wait_ge(wsem, 128 * rho)
                        for k in range(NCORE):
                            rdests = [None] * 8
                            rdests[k] = (0, k)
                            nc.gpsimd.remote_dma_broadcast(
                                out_ap=staging[:, par, k, :ng * 128],
                                in_ap=h_stage[:, g0:g1].rearrange(
                                    "p g f -> p (g f)"),
                                remote_sem=rsem[k], local_sem=lsem,
                                rdests=rdests, queue_num=k % 4,
                            ).then_inc(prep_sem, 1)
                        nc.gpsimd.wait_ge(prep_sem, 8 * (rho + 1))
                        for q in range(4):
                            nc.gpsimd.trigger_dma(count=2, queue_num=q)
                        pid = nc.sync.partition_id()
                        for k in range(NCORE):
                            nc.sync.wait_ge(rsem[k], 2 * (rho + 1))
                            srcc = nc.sync.alloc_register(f"sc{rho}_{k}")
                            nc.sync.reg_alu(srcc, pid, DELTA[k],
                                            mybir.AluOpType.bitwise_xor)
                            srcv = nc.sync.snap(srcc, donate=True,
                                                min_val=0, max_val=7)
                            row0 = srcv * NPC + (g0 * 128)
                            nc.sync.dma_start(
                                out=table_d[bass.ds(row0, ng * 128), :].rearrange(
                                    "(g p) f -> p g f", p=128),
                                in_=staging[:, par, k, :ng * 128].rearrange(
                                    "p (g f) -> p g f", f=LAT),
                            ).then_inc(wsem, 16)
                        rnd[0] += 1
                    nc.sync.wait_ge(wsem, 128 * rnd[0])

            # ---- encoder ----
            scope_enc = nc.named_scope("encoder"); scope_enc.__enter__()
            for b in range(NBLK):
                bs = slice(b * 128, (b + 1) * 128)
                fb = tpool.tile([128, 128], F32, tag="fb")
                nc.scalar.dma_start(out=fb[:], in_=featT_d[:, bs])
                p1 = pdns.tile([128, 128], F32, tag="pd")
                nc.tensor.matmul(out=p1[:], lhsT=benc[:], rhs=ones32[:],
                                 start=True, stop=False)
                nc.tensor.matmul(out=p1[:], lhsT=wenc[:], rhs=fb[:],
                                 start=False, stop=True)
                nc.scalar.activation(out=h_T[:, bs], in_=p1[:],
                                     func=mybir.ActivationFunctionType.Copy)
                pm = pdns.tile([128, 128], F32, tag="pd")
                nc.tensor.matmul(out=pm[:], lhsT=h_T[:, bs], rhs=wn[0][:],
                                 start=True, stop=True)
                nc.scalar.activation(out=h_stage[:, b], in_=pm[:],
                                     func=mybir.ActivationFunctionType.Copy)

            scope_enc.__exit__(None, None, None)
            with nc.named_scope("exchange0"):
                exchange()

            # ---- layers ----
            for l in range(2):
                # segment phase
                cur_psum = None
                for ci, (t0, nt, w) in enumerate(chunks):
                    it = ipool.tile([128, CH_T * 8], I16)
                    nc.scalar.dma_start(out=it[:, :nt * 8],
                                        in_=gidx_d[:, t0 * 8:(t0 + nt) * 8])
                    msg = mpool.tile([128, CH_T, 128], F16)
                    src_ap = table_d[0:LO_ROWS, :] if w == 0 else table_d[LO_ROWS:NTOT, :]
                    nc.gpsimd.dma_gather(
                        out_ap=msg[:, :nt, :], in_ap=src_ap,
                        idxs_ap=it[:, :nt * 8],
                        num_idxs=128 * nt, num_idxs_reg=128 * nt,
                        elem_size=LAT, single_packet=False,
                        queue_num=ci % 4)
                    SW = 4   # S tiles built per DVE op
                    sbuilt = {}
                    for j0 in range(0, nt, SW):
                        jn = min(SW, nt - j0)
                        St = stp.tile([128, SW, 128], F16)
                        nc.vector.tensor_tensor(
                            out=St[:, :jn, :],
                            in0=iota[:].unsqueeze(1).to_broadcast([128, jn, 128]),
                            in1=srcrel[:, t0 + j0:t0 + j0 + jn].to_broadcast(
                                [128, jn, 128]),
                            op=mybir.AluOpType.is_equal)
                        sbuilt[j0] = St
                    for j in range(nt):
                        t = t0 + j
                        b, w_, i = tiles[t]
                        St = sbuilt[j - j % SW]
                        if i == 0:
                            cur_psum = pseg.tile([128, 128], F32, tag="pg")
                        last = (i == T[b, w_] - 1)
                        nc.tensor.matmul(out=cur_psum[:], lhsT=St[:, j % SW, :],
                                         rhs=msg[:, j, :],
                                         start=(i == 0), stop=last)
                        if last:
                            if w_ == 0:
                                nc.vector.tensor_copy(out=agg[:, b], in_=cur_psum[:])
                            else:
                                nc.vector.tensor_tensor(
                                    out=agg[:, b], in0=agg[:, b], in1=cur_psum[:],
                                    op=mybir.AluOpType.add)
                # dense phase
                for b in range(NBLK):
                    bs = slice(b * 128, (b + 1) * 128)
                    pd = pdns.tile([128, 128], F32, tag="pd")
                    nc.tensor.matmul(out=pd[:], lhsT=ones16[:], rhs=bc[l][:],
                                     start=True, stop=False)
                    nc.tensor.matmul(out=pd[:], lhsT=h_T[:, bs], rhs=ws[l][:],
                                     start=False, stop=True)
                    tmp = tpool.tile([128, 128], F32, tag="tmp")
                    nc.scalar.activation(out=tmp[:], in_=agg[:, b],
                                         func=mybir.ActivationFunctionType.Copy,
                                         scale=invd[:, b:b + 1])
                    tmp2 = tpool.tile([128, 128], F32, tag="tmp2")
                    nc.vector.tensor_tensor(out=tmp2[:], in0=tmp[:], in1=pd[:],
                                            op=mybir.AluOpType.add)
                    hn = tpool.tile([128, 128], F16, tag="hn")
                    nc.scalar.activation(out=hn[:], in_=tmp2[:],
                                         func=mybir.ActivationFunctionType.Relu)
                    pt = ptrn.tile([128, 128], F16, tag="pt")
                    nc.tensor.transpose(out=pt[:], in_=hn[:], identity=ident[:])
                    nc.scalar.activation(out=h_T[:, bs], in_=pt[:],
                                         func=mybir.ActivationFunctionType.Copy)
                    if l == 0:
                        pm = pdns.tile([128, 128], F32, tag="pd")
                        nc.tensor.matmul(out=pm[:], lhsT=h_T[:, bs], rhs=wn[1][:],
                                         start=True, stop=True)
                        nc.scalar.activation(out=h_stage[:, b], in_=pm[:],
                                             func=mybir.ActivationFunctionType.Copy)
                if l == 0:
                    with nc.named_scope("exchange1"):
                        exchange()

            # ---- output ----
            for b in range(NBLK):
                bs = slice(b * 128, (b + 1) * 128)
                po = pdns.tile([128, OUT_DIM], F32, tag="pd")
                nc.tensor.matmul(out=po[:], lhsT=ones16[:], rhs=bout[:],
                                 start=True, stop=False)
                nc.tensor.matmul(out=po[:], lhsT=h_T[:, bs], rhs=wout[:],
                                 start=False, stop=True)
                ob = opool.tile([128, OUT_DIM], F32)
                nc.scalar.activation(out=ob[:], in_=po[:],
                                     func=mybir.ActivationFunctionType.Copy)
                nc.sync.dma_start(out=out_d[bs, :], in_=ob[:])

    nc.compile()
    return nc


def make_in_maps(inputs):
    features = np.asarray(inputs["features"], np.float32)
    edge_list = np.asarray(inputs["edge_list"])
    meta, featT, invd, gidx, srcrel = preprocess(features, edge_list)
    w16 = lambda x: np.asarray(x, np.float16)
    in_maps = []
    for c in range(NCORE):
        in_maps.append(dict(
            featT=featT[c], invd=invd[c], gidx=gidx[c], srcrel=srcrel[c],
            wenc=np.asarray(inputs["W_enc"], np.float32),
            benc=np.asarray(inputs["b_enc"], np.float32).reshape(1, LAT),
            ws0=w16(inputs["W_self"][0]), ws1=w16(inputs["W_self"][1]),
            wn0=w16(inputs["W_neigh"][0]), wn1=w16(inputs["W_neigh"][1]),
            bc0=w16(inputs["b_comb"][0]).reshape(1, LAT),
            bc1=w16(inputs["b_comb"][1]).reshape(1, LAT),
            wout=w16(inputs["W_out"]),
            bout=w16(inputs["b_out"]).reshape(1, OUT_DIM),
        ))
    return meta, in_maps


def assemble(results):
    outs = [results[c]["outp"][:NPC_REAL] for c in range(NCORE)]
    return np.concatenate(outs, axis=0)


def kernel(**inputs):
    """Full-input entry point: shard, compile, run on 8 cores, gather."""
    from concourse import bass_utils
    meta, in_maps = make_in_maps(inputs)
    nc = build(meta)
    res = bass_utils.run_bass_kernel_spmd(nc, in_maps, core_ids=list(range(NCORE)))
    return assemble(res.results)



# revision 9
# speedup vs baseline: 1.1868x; 1.0036x over previous
"""GNN message-passing (GraphSAGE-mean style) kernel for 8 TRN2 NeuronCores.

Self-contained: accepts the FULL inputs of the reference problem
(50000 nodes, 800000 edges, 128->128(x2 layers)->64) and returns the full
[50000, 64] output, distributing work across 8 NeuronCores internally.

Approach:
- Nodes sharded 8 ways (6250 + 22 pad = 6272/core, 49 blocks of 128).
- Edges sharded by src owner, grouped by (src block, dst int16-window),
  tile counts equalized across cores so one SPMD NEFF fits all.
- Per layer each core keeps the full fp16 m-table (m = h @ W_neigh) in its
  own HBM; per-edge messages fetched with dma_gather over 4 SWDGE queues.
- Segment-sum via PE matmuls with per-tile indicator matrices built on DVE.
- Dense updates in fp16 on PE with fp32 PSUM; relu/copies on ACT.
- Per-layer cross-core exchange via single-dest remote_dma_broadcast into
  XOR-slot SBUF staging, reordered into the node-ordered HBM table with
  dynamic-offset local DMAs (partition-id XOR register arithmetic).
"""
import numpy as np

import concourse.bacc as bacc
import concourse.bass as bass
import concourse.mybir as mybir
from concourse.tile import TileContext
from concourse.masks import make_identity
from concourse.tile_rust import add_dep_helper

N_NODES = 50000
N_EDGES = 800000
IN_DIM = 128
LAT = 128
OUT_DIM = 64

NCORE = 8
NPC_REAL = N_NODES // NCORE          # 6250
NBLK = 49                            # blocks of 128 per core
NPC = NBLK * 128                     # 6272 padded nodes per core
NTOT = NPC * NCORE                   # 50176 table rows
LO_ROWS = 32768
CH_T = 32                            # tiles per gather chunk
G = 25                               # blocks per exchange round
N_ROUNDS = 2
DELTA = [0, 1, 2, 3, 6, 7, 4, 5]     # HW slot -> delivered tpb delta

F16 = mybir.dt.float16
F32 = mybir.dt.float32
I16 = mybir.dt.int16


def preprocess(features, edge_list):
    src = edge_list[:, 0].astype(np.int64)
    dst = edge_list[:, 1].astype(np.int64)
    cnt = np.bincount(src, minlength=N_NODES).astype(np.float32)
    inv_deg_full = (1.0 / np.maximum(cnt, 1.0)).astype(np.float32)

    tpos = (dst // NPC_REAL) * NPC + (dst % NPC_REAL)
    owner = src // NPC_REAL
    srel_all = src % NPC_REAL
    blk_all = srel_all // 128

    order = np.lexsort((tpos, blk_all, owner))
    so, sb = owner[order], blk_all[order]
    sw = (tpos[order] >= LO_ROWS).astype(np.int64)
    st, sr = tpos[order], srel_all[order]
    key = ((so * NBLK) + sb) * 2 + sw
    bounds = np.searchsorted(key, np.arange(NCORE * NBLK * 2 + 1))
    per = {}
    for c in range(NCORE):
        for b in range(NBLK):
            for w in range(2):
                k = (c * NBLK + b) * 2 + w
                lo, hi = bounds[k], bounds[k + 1]
                per[(c, b, w)] = (st[lo:hi], sr[lo:hi])

    T = np.zeros((NBLK, 2), np.int64)
    for b in range(NBLK):
        for w in range(2):
            mx = max(len(per[(c, b, w)][0]) for c in range(NCORE))
            T[b, w] = max(1, -(-mx // 128))

    tiles = []
    for w in range(2):
        for b in range(NBLK):
            for i in range(T[b, w]):
                tiles.append((b, w, i))
    NT = len(tiles)

    chunks = []
    i = 0
    while i < NT:
        w = tiles[i][1]
        j = i
        while j < NT and tiles[j][1] == w and j - i < CH_T:
            j += 1
        chunks.append((i, j - i, w))
        i = j

    gidx = np.zeros((NCORE, 128, NT * 8), np.int16)
    srcrel = np.full((NCORE, 128, NT), -1.0, np.float16)
    for c in range(NCORE):
        for t, (b, w, i) in enumerate(tiles):
            pos, rel = per[(c, b, w)]
            seg = pos[i * 128:(i + 1) * 128]
            relseg = rel[i * 128:(i + 1) * 128]
            n = len(seg)
            idx = np.zeros(128, np.int64)
            idx[:n] = seg - (LO_ROWS if w else 0)
            col = np.full(128, -1.0, np.float32)
            col[:n] = relseg - b * 128
            a = idx.astype(np.int16).reshape(8, 16).T
            gidx[c, :, t * 8:(t + 1) * 8] = np.tile(a, (8, 1))
            srcrel[c, :, t] = col.astype(np.float16)

    featT = np.zeros((NCORE, 128, NPC), np.float32)
    invd = np.ones((NCORE, 128, NBLK), np.float32)
    for c in range(NCORE):
        f = features[c * NPC_REAL:(c + 1) * NPC_REAL]
        featT[c, :, :NPC_REAL] = f.T
        col = np.ones(NPC, np.float32)
        col[:NPC_REAL] = inv_deg_full[c * NPC_REAL:(c + 1) * NPC_REAL]
        invd[c] = col.reshape(NBLK, 128).T

    meta = dict(tiles=tiles, chunks=chunks, T=T, NT=NT)
    return meta, featT, invd, gidx, srcrel


def build(meta):
    tiles, chunks, T, NT = meta["tiles"], meta["chunks"], meta["T"], meta["NT"]
    nc = bacc.Bacc("TRN2", target_bir_lowering=False, debug=False,
                   enable_asserts=True, num_devices=NCORE, num_swdge_queues=4)

    featT_d = nc.dram_tensor("featT", [128, NPC], F32, kind="ExternalInput")
    wenc_d = nc.dram_tensor("wenc", [IN_DIM, LAT], F32, kind="ExternalInput")
    benc_d = nc.dram_tensor("benc", [1, LAT], F32, kind="ExternalInput")
    ws_d = [nc.dram_tensor(f"ws{l}", [LAT, LAT], F16, kind="ExternalInput") for l in range(2)]
    wn_d = [nc.dram_tensor(f"wn{l}", [LAT, LAT], F16, kind="ExternalInput") for l in range(2)]
    bc_d = [nc.dram_tensor(f"bc{l}", [1, LAT], F16, kind="ExternalInput") for l in range(2)]
    wout_d = nc.dram_tensor("wout", [LAT, OUT_DIM], F16, kind="ExternalInput")
    bout_d = nc.dram_tensor("bout", [1, OUT_DIM], F16, kind="ExternalInput")
    invd_d = nc.dram_tensor("invd", [128, NBLK], F32, kind="ExternalInput")
    gidx_d = nc.dram_tensor("gidx", [128, NT * 8], I16, kind="ExternalInput")
    srcrel_d = nc.dram_tensor("srcrel", [128, NT], F16, kind="ExternalInput")
    out_d = nc.dram_tensor("outp", [NPC, OUT_DIM], F32, kind="ExternalOutput")
    table_d = nc.dram_tensor("table", [NTOT, LAT], F16, kind="Internal")

    with TileContext(nc) as tc:
        with tc.tile_pool(name="consts", bufs=1) as cpool, \
             tc.tile_pool(name="agg", bufs=1) as apool, \
             tc.tile_pool(name="ht", bufs=1) as hpool, \
             tc.tile_pool(name="stage", bufs=1) as spool, \
             tc.tile_pool(name="msg", bufs=4) as mpool, \
             tc.tile_pool(name="idx", bufs=6) as ipool, \
             tc.tile_pool(name="stile", bufs=6) as stp, \
             tc.tile_pool(name="tmp", bufs=4) as tpool, \
             tc.tile_pool(name="outb", bufs=3) as opool, \
             tc.tile_pool(name="pseg", bufs=4, space="PSUM") as pseg, \
             tc.tile_pool(name="pdns", bufs=3, space="PSUM") as pdns, \
             tc.tile_pool(name="ptrn", bufs=1, space="PSUM") as ptrn:

            # ---- constants ----
            wenc = cpool.tile([IN_DIM, LAT], F32)
            nc.sync.dma_start(out=wenc[:], in_=wenc_d[:])
            benc = cpool.tile([1, LAT], F32)
            nc.sync.dma_start(out=benc[:], in_=benc_d[:])
            ws, wn, bc = [], [], []
            for l in range(2):
                w_ = cpool.tile([LAT, LAT], F16, tag=f"ws{l}")
                nc.sync.dma_start(out=w_[:], in_=ws_d[l][:])
                ws.append(w_)
                w2 = cpool.tile([LAT, LAT], F16, tag=f"wn{l}")
                nc.sync.dma_start(out=w2[:], in_=wn_d[l][:])
                wn.append(w2)
                b_ = cpool.tile([1, LAT], F16, tag=f"bc{l}")
                nc.sync.dma_start(out=b_[:], in_=bc_d[l][:])
                bc.append(b_)
            wout = cpool.tile([LAT, OUT_DIM], F16)
            nc.sync.dma_start(out=wout[:], in_=wout_d[:])
            bout = cpool.tile([1, OUT_DIM], F16)
            nc.sync.dma_start(out=bout[:], in_=bout_d[:])
            invd = cpool.tile([128, NBLK], F32)
            nc.sync.dma_start(out=invd[:], in_=invd_d[:])
            srcrel = cpool.tile([128, NT], F16)
            nc.gpsimd.dma_start(out=srcrel[:], in_=srcrel_d[:])
            ones16 = cpool.tile([1, 128], F16)
            nc.vector.memset(ones16[:], 1.0)
            ones32 = cpool.tile([1, 128], F32)
            nc.vector.memset(ones32[:], 1.0)
            iota32 = cpool.tile([128, 128], F32)
            nc.gpsimd.iota(iota32[:], [[1, 128]], base=0, channel_multiplier=0,
                           allow_small_or_imprecise_dtypes=True)
            iota = cpool.tile([128, 128], F16)
            nc.vector.tensor_copy(out=iota[:], in_=iota32[:])
            ident = cpool.tile([128, 128], F16)
            make_identity(nc, ident[:])

            agg = apool.tile([128, NBLK, 128], F32)
            h_T = hpool.tile([128, NPC], F16)
            h_stage = spool.tile([128, NBLK, 128], F16)
            staging = spool.tile([128, 2, NCORE, G * 128], F16)

            gsem = [nc.alloc_semaphore(f"gsem{q}") for q in range(4)]
            rsem = [nc.alloc_semaphore(f"rsem{k}") for k in range(NCORE)]
            lsem = nc.alloc_semaphore("lsem")
            prep_sem = nc.alloc_semaphore("prep_sem")
            wsem = nc.alloc_semaphore("wsem")
            rnd = [0]

            with tc.tile_critical():
                nc.gpsimd.bir_kernel_barrier_wait([[i for i in range(NCORE)]])

            def exchange():
                with tc.tile_critical():
                    for r in range(N_ROUNDS):
                        g0 = r * G
                        g1 = min(NBLK, g0 + G)
                        ng = g1 - g0
                        rho = rnd[0]
                        par = rho % 2
                        if rho != 1:
                            if rho > 0:
                                nc.gpsimd.wait_ge(wsem, 128 * rho)
                        for k in range(NCORE):
                            rdests = [None] * 8
                            rdests[k] = (0, k)
                            nc.gpsimd.remote_dma_broadcast(
                                out_ap=staging[:, par, k, :ng * 128],
                                in_ap=h_stage[:, g0:g1].rearrange(
                                    "p g f -> p (g f)"),
                                remote_sem=rsem[k], local_sem=lsem,
                                rdests=rdests, queue_num=k % 4,
                            ).then_inc(prep_sem, 1)
                        nc.gpsimd.wait_ge(prep_sem, 8 * (rho + 1))
                        for q in range(4):
                            nc.gpsimd.trigger_dma(count=2, queue_num=q)
                        pid = nc.sync.partition_id()
                        for k in range(NCORE):
                            nc.sync.wait_ge(rsem[k], 2 * (rho + 1))
                            srcc = nc.sync.alloc_register(f"sc{rho}_{k}")
                            nc.sync.reg_alu(srcc, pid, DELTA[k],
                                            mybir.AluOpType.bitwise_xor)
                            srcv = nc.sync.snap(srcc, donate=True,
                                                min_val=0, max_val=7)
                            row0 = srcv * NPC + (g0 * 128)
                            nc.sync.dma_start(
                                out=table_d[bass.ds(row0, ng * 128), :].rearrange(
                                    "(g p) f -> p g f", p=128),
                                in_=staging[:, par, k, :ng * 128].rearrange(
                                    "p (g f) -> p g f", f=LAT),
                            ).then_inc(wsem, 16)
                        rnd[0] += 1
                    nc.sync.wait_ge(wsem, 128 * rnd[0])

            # ---- encoder ----
            scope_enc = nc.named_scope("encoder"); scope_enc.__enter__()
            for b in range(NBLK):
                bs = slice(b * 128, (b + 1) * 128)
                fb = tpool.tile([128, 128], F32, tag="fb")
                nc.scalar.dma_start(out=fb[:], in_=featT_d[:, bs])
                p1 = pdns.tile([128, 128], F32, tag="pd")
                nc.tensor.matmul(out=p1[:], lhsT=benc[:], rhs=ones32[:],
                                 start=True, stop=False)
                nc.tensor.matmul(out=p1[:], lhsT=wenc[:], rhs=fb[:],
                                 start=False, stop=True)
                nc.scalar.activation(out=h_T[:, bs], in_=p1[:],
                                     func=mybir.ActivationFunctionType.Copy)
                pm = pdns.tile([128, 128], F32, tag="pd")
                nc.tensor.matmul(out=pm[:], lhsT=h_T[:, bs], rhs=wn[0][:],
                                 start=True, stop=True)
                nc.scalar.activation(out=h_stage[:, b], in_=pm[:],
                                     func=mybir.ActivationFunctionType.Copy)

            scope_enc.__exit__(None, None, None)
            with nc.named_scope("exchange0"):
                exchange()

            # ---- layers ----
            for l in range(2):
                # segment phase
                cur_psum = None
                for ci, (t0, nt, w) in enumerate(chunks):
                    it = ipool.tile([128, CH_T * 8], I16)
                    nc.scalar.dma_start(out=it[:, :nt * 8],
                                        in_=gidx_d[:, t0 * 8:(t0 + nt) * 8])
                    msg = mpool.tile([128, CH_T, 128], F16)
                    src_ap = table_d[0:LO_ROWS, :] if w == 0 else table_d[LO_ROWS:NTOT, :]
                    nc.gpsimd.dma_gather(
                        out_ap=msg[:, :nt, :], in_ap=src_ap,
                        idxs_ap=it[:, :nt * 8],
                        num_idxs=128 * nt, num_idxs_reg=128 * nt,
                        elem_size=LAT, single_packet=False,
                        queue_num=ci % 4)
                    SW = 4   # S tiles built per DVE op
                    sbuilt = {}
                    for j0 in range(0, nt, SW):
                        jn = min(SW, nt - j0)
                        St = stp.tile([128, SW, 128], F16)
                        nc.vector.tensor_tensor(
                            out=St[:, :jn, :],
                            in0=iota[:].unsqueeze(1).to_broadcast([128, jn, 128]),
                            in1=srcrel[:, t0 + j0:t0 + j0 + jn].to_broadcast(
                                [128, jn, 128]),
                            op=mybir.AluOpType.is_equal)
                        sbuilt[j0] = St
                    for j in range(nt):
                        t = t0 + j
                        b, w_, i = tiles[t]
                        St = sbuilt[j - j % SW]
                        if i == 0:
                            cur_psum = pseg.tile([128, 128], F32, tag="pg")
                        last = (i == T[b, w_] - 1)
                        nc.tensor.matmul(out=cur_psum[:], lhsT=St[:, j % SW, :],
                                         rhs=msg[:, j, :],
                                         start=(i == 0), stop=last)
                        if last:
                            if w_ == 0:
                                nc.vector.tensor_copy(out=agg[:, b], in_=cur_psum[:])
                            else:
                                nc.vector.tensor_tensor(
                                    out=agg[:, b], in0=agg[:, b], in1=cur_psum[:],
                                    op=mybir.AluOpType.add)
                # dense phase
                for b in range(NBLK):
                    bs = slice(b * 128, (b + 1) * 128)
                    pd = pdns.tile([128, 128], F32, tag="pd")
                    nc.tensor.matmul(out=pd[:], lhsT=ones16[:], rhs=bc[l][:],
                                     start=True, stop=False)
                    nc.tensor.matmul(out=pd[:], lhsT=h_T[:, bs], rhs=ws[l][:],
                                     start=False, stop=True)
                    tmp = tpool.tile([128, 128], F32, tag="tmp")
                    nc.scalar.activation(out=tmp[:], in_=agg[:, b],
                                         func=mybir.ActivationFunctionType.Copy,
                                         scale=invd[:, b:b + 1])
                    tmp2 = tpool.tile([128, 128], F32, tag="tmp2")
                    nc.vector.tensor_tensor(out=tmp2[:], in0=tmp[:], in1=pd[:],
                                            op=mybir.AluOpType.add)
                    hn = tpool.tile([128, 128], F16, tag="hn")
                    nc.scalar.activation(out=hn[:], in_=tmp2[:],
                                         func=mybir.ActivationFunctionType.Relu)
                    pt = ptrn.tile([128, 128], F16, tag="pt")
                    nc.tensor.transpose(out=pt[:], in_=hn[:], identity=ident[:])
                    nc.scalar.activation(out=h_T[:, bs], in_=pt[:],
                                         func=mybir.ActivationFunctionType.Copy)
                    if l == 0:
                        pm = pdns.tile([128, 128], F32, tag="pd")
                        nc.tensor.matmul(out=pm[:], lhsT=h_T[:, bs], rhs=wn[1][:],
                                         start=True, stop=True)
                        nc.scalar.activation(out=h_stage[:, b], in_=pm[:],
                                             func=mybir.ActivationFunctionType.Copy)
                if l == 0:
                    with nc.named_scope("exchange1"):
                        exchange()

            # ---- output ----
            for b in range(NBLK):
                bs = slice(b * 128, (b + 1) * 128)
                po = pdns.tile([128, OUT_DIM], F32, tag="pd")
                nc.tensor.matmul(out=po[:], lhsT=ones16[:], rhs=bout[:],
                                 start=True, stop=False)
                nc.tensor.matmul(out=po[:], lhsT=h_T[:, bs], rhs=wout[:],
                                 start=False, stop=True)
                ob = opool.tile([128, OUT_DIM], F32)
                nc.scalar.activation(out=ob[:], in_=po[:],
                                     func=mybir.ActivationFunctionType.Copy)
                nc.sync.dma_start(out=out_d[bs, :], in_=ob[:])

    nc.compile()
    return nc


def make_in_maps(inputs):
    features = np.asarray(inputs["features"], np.float32)
    edge_list = np.asarray(inputs["edge_list"])
    meta, featT, invd, gidx, srcrel = preprocess(features, edge_list)
    w16 = lambda x: np.asarray(x, np.float16)
    in_maps = []
    for c in range(NCORE):
        in_maps.append(dict(
            featT=featT[c], invd=invd[c], gidx=gidx[c], srcrel=srcrel[c],
            wenc=np.asarray(inputs["W_enc"], np.float32),
            benc=np.asarray(inputs["b_enc"], np.float32).reshape(1, LAT),
            ws0=w16(inputs["W_self"][0]), ws1=w16(inputs["W_self"][1]),
            wn0=w16(inputs["W_neigh"][0]), wn1=w16(inputs["W_neigh"][1]),
            bc0=w16(inputs["b_comb"][0]).reshape(1, LAT),
            bc1=w16(inputs["b_comb"][1]).reshape(1, LAT),
            wout=w16(inputs["W_out"]),
            bout=w16(inputs["b_out"]).reshape(1, OUT_DIM),
        ))
    return meta, in_maps


def assemble(results):
    outs = [results[c]["outp"][:NPC_REAL] for c in range(NCORE)]
    return np.concatenate(outs, axis=0)


def kernel(**inputs):
    """Full-input entry point: shard, compile, run on 8 cores, gather."""
    from concourse import bass_utils
    meta, in_maps = make_in_maps(inputs)
    nc = build(meta)
    res = bass_utils.run_bass_kernel_spmd(nc, in_maps, core_ids=list(range(NCORE)))
    return assemble(res.results)



# revision 10
# speedup vs baseline: 1.1981x; 1.0095x over previous
"""GNN message-passing (GraphSAGE-mean style) kernel for 8 TRN2 NeuronCores.

Self-contained: accepts the FULL inputs of the reference problem
(50000 nodes, 800000 edges, 128->128(x2 layers)->64) and returns the full
[50000, 64] output, distributing work across 8 NeuronCores internally.

Approach:
- Nodes sharded 8 ways (6250 + 22 pad = 6272/core, 49 blocks of 128).
- Edges sharded by src owner, grouped by (src block, dst int16-window),
  tile counts equalized across cores so one SPMD NEFF fits all.
- Per layer each core keeps the full fp16 m-table (m = h @ W_neigh) in its
  own HBM; per-edge messages fetched with dma_gather over 4 SWDGE queues.
- Segment-sum via PE matmuls with per-tile indicator matrices built on DVE.
- Dense updates in fp16 on PE with fp32 PSUM; relu/copies on ACT.
- Per-layer cross-core exchange via single-dest remote_dma_broadcast into
  XOR-slot SBUF staging, reordered into the node-ordered HBM table with
  dynamic-offset local DMAs (partition-id XOR register arithmetic).
"""
import numpy as np

import concourse.bacc as bacc
import concourse.bass as bass
import concourse.mybir as mybir
from concourse.tile import TileContext
from concourse.masks import make_identity
from concourse.tile_rust import add_dep_helper

N_NODES = 50000
N_EDGES = 800000
IN_DIM = 128
LAT = 128
OUT_DIM = 64

NCORE = 8
NPC_REAL = N_NODES // NCORE          # 6250
NBLK = 49                            # blocks of 128 per core
NPC = NBLK * 128                     # 6272 padded nodes per core
NTOT = NPC * NCORE                   # 50176 table rows
LO_ROWS = 32768
CH_T = 32                            # tiles per gather chunk
G = 25                               # blocks per exchange round
N_ROUNDS = 2
DELTA = [0, 1, 2, 3, 6, 7, 4, 5]     # HW slot -> delivered tpb delta

F16 = mybir.dt.float16
F32 = mybir.dt.float32
I16 = mybir.dt.int16


def preprocess(features, edge_list):
    src = edge_list[:, 0].astype(np.int64)
    dst = edge_list[:, 1].astype(np.int64)
    cnt = np.bincount(src, minlength=N_NODES).astype(np.float32)
    inv_deg_full = (1.0 / np.maximum(cnt, 1.0)).astype(np.float32)

    tpos = (dst // NPC_REAL) * NPC + (dst % NPC_REAL)
    owner = src // NPC_REAL
    srel_all = src % NPC_REAL
    blk_all = srel_all // 128

    order = np.lexsort((tpos, blk_all, owner))
    so, sb = owner[order], blk_all[order]
    sw = (tpos[order] >= LO_ROWS).astype(np.int64)
    st, sr = tpos[order], srel_all[order]
    key = ((so * NBLK) + sb) * 2 + sw
    bounds = np.searchsorted(key, np.arange(NCORE * NBLK * 2 + 1))
    per = {}
    for c in range(NCORE):
        for b in range(NBLK):
            for w in range(2):
                k = (c * NBLK + b) * 2 + w
                lo, hi = bounds[k], bounds[k + 1]
                per[(c, b, w)] = (st[lo:hi], sr[lo:hi])

    T = np.zeros((NBLK, 2), np.int64)
    for b in range(NBLK):
        for w in range(2):
            mx = max(len(per[(c, b, w)][0]) for c in range(NCORE))
            T[b, w] = max(1, -(-mx // 128))

    tiles = []
    for w in range(2):
        for b in range(NBLK):
            for i in range(T[b, w]):
                tiles.append((b, w, i))
    NT = len(tiles)

    chunks = []
    i = 0
    while i < NT:
        w = tiles[i][1]
        j = i
        while j < NT and tiles[j][1] == w and j - i < CH_T:
            j += 1
        chunks.append((i, j - i, w))
        i = j

    gidx = np.zeros((NCORE, 128, NT * 8), np.int16)
    srcrel = np.full((NCORE, 128, NT), -1.0, np.float16)
    for c in range(NCORE):
        for t, (b, w, i) in enumerate(tiles):
            pos, rel = per[(c, b, w)]
            seg = pos[i * 128:(i + 1) * 128]
            relseg = rel[i * 128:(i + 1) * 128]
            n = len(seg)
            idx = np.zeros(128, np.int64)
            idx[:n] = seg - (LO_ROWS if w else 0)
            col = np.full(128, -1.0, np.float32)
            col[:n] = relseg - b * 128
            a = idx.astype(np.int16).reshape(8, 16).T
            gidx[c, :, t * 8:(t + 1) * 8] = np.tile(a, (8, 1))
            srcrel[c, :, t] = col.astype(np.float16)

    featT = np.zeros((NCORE, 128, NPC), np.float32)
    invd = np.ones((NCORE, 128, NBLK), np.float32)
    for c in range(NCORE):
        f = features[c * NPC_REAL:(c + 1) * NPC_REAL]
        featT[c, :, :NPC_REAL] = f.T
        col = np.ones(NPC, np.float32)
        col[:NPC_REAL] = inv_deg_full[c * NPC_REAL:(c + 1) * NPC_REAL]
        invd[c] = col.reshape(NBLK, 128).T

    meta = dict(tiles=tiles, chunks=chunks, T=T, NT=NT)
    return meta, featT, invd, gidx, srcrel


def build(meta):
    tiles, chunks, T, NT = meta["tiles"], meta["chunks"], meta["T"], meta["NT"]
    nc = bacc.Bacc("TRN2", target_bir_lowering=False, debug=False,
                   enable_asserts=True, num_devices=NCORE, num_swdge_queues=4)

    featT_d = nc.dram_tensor("featT", [128, NPC], F32, kind="ExternalInput")
    wenc_d = nc.dram_tensor("wenc", [IN_DIM, LAT], F32, kind="ExternalInput")
    benc_d = nc.dram_tensor("benc", [1, LAT], F32, kind="ExternalInput")
    ws_d = [nc.dram_tensor(f"ws{l}", [LAT, LAT], F16, kind="ExternalInput") for l in range(2)]
    wn_d = [nc.dram_tensor(f"wn{l}", [LAT, LAT], F16, kind="ExternalInput") for l in range(2)]
    bc_d = [nc.dram_tensor(f"bc{l}", [1, LAT], F16, kind="ExternalInput") for l in range(2)]
    wout_d = nc.dram_tensor("wout", [LAT, OUT_DIM], F16, kind="ExternalInput")
    bout_d = nc.dram_tensor("bout", [1, OUT_DIM], F16, kind="ExternalInput")
    invd_d = nc.dram_tensor("invd", [128, NBLK], F32, kind="ExternalInput")
    gidx_d = nc.dram_tensor("gidx", [128, NT * 8], I16, kind="ExternalInput")
    srcrel_d = nc.dram_tensor("srcrel", [128, NT], F16, kind="ExternalInput")
    out_d = nc.dram_tensor("outp", [NPC, OUT_DIM], F32, kind="ExternalOutput")
    table_d = nc.dram_tensor("table", [NTOT, LAT], F16, kind="Internal")

    with TileContext(nc) as tc:
        with tc.tile_pool(name="consts", bufs=1) as cpool, \
             tc.tile_pool(name="agg", bufs=1) as apool, \
             tc.tile_pool(name="ht", bufs=1) as hpool, \
             tc.tile_pool(name="stage", bufs=1) as spool, \
             tc.tile_pool(name="msg", bufs=4) as mpool, \
             tc.tile_pool(name="idx", bufs=6) as ipool, \
             tc.tile_pool(name="stile", bufs=3) as stp, \
             tc.tile_pool(name="tmp", bufs=4) as tpool, \
             tc.tile_pool(name="outb", bufs=3) as opool, \
             tc.tile_pool(name="pseg", bufs=4, space="PSUM") as pseg, \
             tc.tile_pool(name="pdns", bufs=3, space="PSUM") as pdns, \
             tc.tile_pool(name="ptrn", bufs=1, space="PSUM") as ptrn:

            # ---- constants ----
            wenc = cpool.tile([IN_DIM, LAT], F32)
            nc.sync.dma_start(out=wenc[:], in_=wenc_d[:])
            benc = cpool.tile([1, LAT], F32)
            nc.sync.dma_start(out=benc[:], in_=benc_d[:])
            ws, wn, bc = [], [], []
            for l in range(2):
                w_ = cpool.tile([LAT, LAT], F16, tag=f"ws{l}")
                nc.sync.dma_start(out=w_[:], in_=ws_d[l][:])
                ws.append(w_)
                w2 = cpool.tile([LAT, LAT], F16, tag=f"wn{l}")
                nc.sync.dma_start(out=w2[:], in_=wn_d[l][:])
                wn.append(w2)
                b_ = cpool.tile([1, LAT], F16, tag=f"bc{l}")
                nc.sync.dma_start(out=b_[:], in_=bc_d[l][:])
                bc.append(b_)
            wout = cpool.tile([LAT, OUT_DIM], F16)
            nc.sync.dma_start(out=wout[:], in_=wout_d[:])
            bout = cpool.tile([1, OUT_DIM], F16)
            nc.sync.dma_start(out=bout[:], in_=bout_d[:])
            invd = cpool.tile([128, NBLK], F32)
            nc.sync.dma_start(out=invd[:], in_=invd_d[:])
            srcrel = cpool.tile([128, NT], F16)
            nc.gpsimd.dma_start(out=srcrel[:], in_=srcrel_d[:])
            ones16 = cpool.tile([1, 128], F16)
            nc.vector.memset(ones16[:], 1.0)
            ones32 = cpool.tile([1, 128], F32)
            nc.vector.memset(ones32[:], 1.0)
            iota32 = cpool.tile([128, 128], F32)
            nc.gpsimd.iota(iota32[:], [[1, 128]], base=0, channel_multiplier=0,
                           allow_small_or_imprecise_dtypes=True)
            iota = cpool.tile([128, 128], F16)
            nc.vector.tensor_copy(out=iota[:], in_=iota32[:])
            ident = cpool.tile([128, 128], F16)
            make_identity(nc, ident[:])

            agg = apool.tile([128, NBLK, 128], F32)
            h_T = hpool.tile([128, NPC], F16)
            h_stage = spool.tile([128, NBLK, 128], F16)
            staging = spool.tile([128, 2, NCORE, G * 128], F16)

            gsem = [nc.alloc_semaphore(f"gsem{q}") for q in range(4)]
            rsem = [nc.alloc_semaphore(f"rsem{k}") for k in range(NCORE)]
            lsem = nc.alloc_semaphore("lsem")
            prep_sem = nc.alloc_semaphore("prep_sem")
            wsem = nc.alloc_semaphore("wsem")
            rnd = [0]

            with tc.tile_critical():
                nc.gpsimd.bir_kernel_barrier_wait([[i for i in range(NCORE)]])

            def exchange():
                with tc.tile_critical():
                    for r in range(N_ROUNDS):
                        g0 = r * G
                        g1 = min(NBLK, g0 + G)
                        ng = g1 - g0
                        rho = rnd[0]
                        par = rho % 2
                        if rho != 1:
                            if rho > 0:
                                nc.gpsimd.wait_ge(wsem, 128 * rho)
                        for k in range(NCORE):
                            rdests = [None] * 8
                            rdests[k] = (0, k)
                            nc.gpsimd.remote_dma_broadcast(
                                out_ap=staging[:, par, k, :ng * 128],
                                in_ap=h_stage[:, g0:g1].rearrange(
                                    "p g f -> p (g f)"),
                                remote_sem=rsem[k], local_sem=lsem,
                                rdests=rdests, queue_num=k % 4,
                            ).then_inc(prep_sem, 1)
                        nc.gpsimd.wait_ge(prep_sem, 8 * (rho + 1))
                        for q in range(4):
                            nc.gpsimd.trigger_dma(count=2, queue_num=q)
                        pid = nc.sync.partition_id()
                        for k in range(NCORE):
                            nc.sync.wait_ge(rsem[k], 2 * (rho + 1))
                            srcc = nc.sync.alloc_register(f"sc{rho}_{k}")
                            nc.sync.reg_alu(srcc, pid, DELTA[k],
                                            mybir.AluOpType.bitwise_xor)
                            srcv = nc.sync.snap(srcc, donate=True,
                                                min_val=0, max_val=7)
                            row0 = srcv * NPC + (g0 * 128)
                            nc.sync.dma_start(
                                out=table_d[bass.ds(row0, ng * 128), :].rearrange(
                                    "(g p) f -> p g f", p=128),
                                in_=staging[:, par, k, :ng * 128].rearrange(
                                    "p (g f) -> p g f", f=LAT),
                            ).then_inc(wsem, 16)
                        rnd[0] += 1
                    nc.sync.wait_ge(wsem, 128 * rnd[0])

            # ---- encoder ----
            scope_enc = nc.named_scope("encoder"); scope_enc.__enter__()
            for b in range(NBLK):
                bs = slice(b * 128, (b + 1) * 128)
                fb = tpool.tile([128, 128], F32, tag="fb")
                nc.scalar.dma_start(out=fb[:], in_=featT_d[:, bs])
                p1 = pdns.tile([128, 128], F32, tag="pd")
                nc.tensor.matmul(out=p1[:], lhsT=benc[:], rhs=ones32[:],
                                 start=True, stop=False)
                nc.tensor.matmul(out=p1[:], lhsT=wenc[:], rhs=fb[:],
                                 start=False, stop=True)
                nc.scalar.activation(out=h_T[:, bs], in_=p1[:],
                                     func=mybir.ActivationFunctionType.Copy)
                pm = pdns.tile([128, 128], F32, tag="pd")
                nc.tensor.matmul(out=pm[:], lhsT=h_T[:, bs], rhs=wn[0][:],
                                 start=True, stop=True)
                nc.scalar.activation(out=h_stage[:, b], in_=pm[:],
                                     func=mybir.ActivationFunctionType.Copy)

            scope_enc.__exit__(None, None, None)
            with nc.named_scope("exchange0"):
                exchange()

            # ---- layers ----
            for l in range(2):
                # segment phase
                cur_psum = None
                for ci, (t0, nt, w) in enumerate(chunks):
                    it = ipool.tile([128, CH_T * 8], I16)
                    nc.scalar.dma_start(out=it[:, :nt * 8],
                                        in_=gidx_d[:, t0 * 8:(t0 + nt) * 8])
                    msg = mpool.tile([128, CH_T, 128], F16)
                    src_ap = table_d[0:LO_ROWS, :] if w == 0 else table_d[LO_ROWS:NTOT, :]
                    nc.gpsimd.dma_gather(
                        out_ap=msg[:, :nt, :], in_ap=src_ap,
                        idxs_ap=it[:, :nt * 8],
                        num_idxs=128 * nt, num_idxs_reg=128 * nt,
                        elem_size=LAT, single_packet=False,
                        queue_num=ci % 4)
                    SW = 8   # S tiles built per DVE op
                    sbuilt = {}
                    for j0 in range(0, nt, SW):
                        jn = min(SW, nt - j0)
                        St = stp.tile([128, SW, 128], F16)
                        nc.vector.tensor_tensor(
                            out=St[:, :jn, :],
                            in0=iota[:].unsqueeze(1).to_broadcast([128, jn, 128]),
                            in1=srcrel[:, t0 + j0:t0 + j0 + jn].to_broadcast(
                                [128, jn, 128]),
                            op=mybir.AluOpType.is_equal)
                        sbuilt[j0] = St
                    for j in range(nt):
                        t = t0 + j
                        b, w_, i = tiles[t]
                        St = sbuilt[j - j % SW]
                        if i == 0:
                            cur_psum = pseg.tile([128, 128], F32, tag="pg")
                        last = (i == T[b, w_] - 1)
                        nc.tensor.matmul(out=cur_psum[:], lhsT=St[:, j % SW, :],
                                         rhs=msg[:, j, :],
                                         start=(i == 0), stop=last)
                        if last:
                            if w_ == 0:
                                nc.vector.tensor_copy(out=agg[:, b], in_=cur_psum[:])
                            else:
                                nc.vector.tensor_tensor(
                                    out=agg[:, b], in0=agg[:, b], in1=cur_psum[:],
                                    op=mybir.AluOpType.add)
                # dense phase
                for b in range(NBLK):
                    bs = slice(b * 128, (b + 1) * 128)
                    pd = pdns.tile([128, 128], F32, tag="pd")
                    nc.tensor.matmul(out=pd[:], lhsT=ones16[:], rhs=bc[l][:],
                                     start=True, stop=False)
                    nc.tensor.matmul(out=pd[:], lhsT=h_T[:, bs], rhs=ws[l][:],
                                     start=False, stop=True)
                    tmp = tpool.tile([128, 128], F32, tag="tmp")
                    nc.scalar.activation(out=tmp[:], in_=agg[:, b],
                                         func=mybir.ActivationFunctionType.Copy,
                                         scale=invd[:, b:b + 1])
                    tmp2 = tpool.tile([128, 128], F32, tag="tmp2")
                    nc.vector.tensor_tensor(out=tmp2[:], in0=tmp[:], in1=pd[:],
                                            op=mybir.AluOpType.add)
                    hn = tpool.tile([128, 128], F16, tag="hn")
                    nc.scalar.activation(out=hn[:], in_=tmp2[:],
                                         func=mybir.ActivationFunctionType.Relu)
                    pt = ptrn.tile([128, 128], F16, tag="pt")
                    nc.tensor.transpose(out=pt[:], in_=hn[:], identity=ident[:])
                    nc.scalar.activation(out=h_T[:, bs], in_=pt[:],
                                         func=mybir.ActivationFunctionType.Copy)
                    if l == 0:
                        pm = pdns.tile([128, 128], F32, tag="pd")
                        nc.tensor.matmul(out=pm[:], lhsT=h_T[:, bs], rhs=wn[1][:],
                                         start=True, stop=True)
                        nc.scalar.activation(out=h_stage[:, b], in_=pm[:],
                                             func=mybir.ActivationFunctionType.Copy)
                if l == 0:
                    with nc.named_scope("exchange1"):
                        exchange()

            # ---- output ----
            for b in range(NBLK):
                bs = slice(b * 128, (b + 1) * 128)
                po = pdns.tile([128, OUT_DIM], F32, tag="pd")
                nc.tensor.matmul(out=po[:], lhsT=ones16[:], rhs=bout[:],
                                 start=True, stop=False)
                nc.tensor.matmul(out=po[:], lhsT=h_T[:, bs], rhs=wout[:],
                                 start=False, stop=True)
                ob = opool.tile([128, OUT_DIM], F32)
                nc.scalar.activation(out=ob[:], in_=po[:],
                                     func=mybir.ActivationFunctionType.Copy)
                nc.sync.dma_start(out=out_d[bs, :], in_=ob[:])

    nc.compile()
    return nc


def make_in_maps(inputs):
    features = np.asarray(inputs["features"], np.float32)
    edge_list = np.asarray(inputs["edge_list"])
    meta, featT, invd, gidx, srcrel = preprocess(features, edge_list)
    w16 = lambda x: np.asarray(x, np.float16)
    in_maps = []
    for c in range(NCORE):
        in_maps.append(dict(
            featT=featT[c], invd=invd[c], gidx=gidx[c], srcrel=srcrel[c],
            wenc=np.asarray(inputs["W_enc"], np.float32),
            benc=np.asarray(inputs["b_enc"], np.float32).reshape(1, LAT),
            ws0=w16(inputs["W_self"][0]), ws1=w16(inputs["W_self"][1]),
            wn0=w16(inputs["W_neigh"][0]), wn1=w16(inputs["W_neigh"][1]),
            bc0=w16(inputs["b_comb"][0]).reshape(1, LAT),
            bc1=w16(inputs["b_comb"][1]).reshape(1, LAT),
            wout=w16(inputs["W_out"]),
            bout=w16(inputs["b_out"]).reshape(1, OUT_DIM),
        ))
    return meta, in_maps


def assemble(results):
    outs = [results[c]["outp"][:NPC_REAL] for c in range(NCORE)]
    return np.concatenate(outs, axis=0)


def kernel(**inputs):
    """Full-input entry point: shard, compile, run on 8 cores, gather."""
    from concourse import bass_utils
    meta, in_maps = make_in_maps(inputs)
    nc = build(meta)
    res = bass_utils.run_bass_kernel_spmd(nc, in_maps, core_ids=list(range(NCORE)))
    return assemble(res.results)

